# revision 12
# baseline (speedup 1.0000x reference)
"""Binarized 3-layer MLP on 8 TRN2 NeuronCores (data-parallel over batch).

Computation (matching the reference):
    h1  = x @ sign(W1).T          x: [65536, 784] fp32, W1: [400, 784]
    h2  = sign(h1) @ sign(W2).T   W2: [200, 400]
    out = sign(h2) @ sign(W3).T   W3: [10, 200]

Strategy (v3):
  - Batch sharded 8192 rows/core; weights replicated. Activations kept
    feature-major (features on SBUF partitions) so every layer's contraction
    dim is already on partitions: no transposes.
  - Layer 1 splits x into hi = fp16(x) plus an fp8 residual:
    lo8 = e4m3((x - hi) * 2^6), matched with fp8 weights sign(W1)*2^-6
    (the min-normal e4m3 value, exact). Products are exact power-of-two
    rescales of the residual, accumulated in the same fp32 PSUM group as the
    fp16 hi matmuls, so h1 carries ~2^-15 relative error -- measured 62 h1
    sign flips over 26M entries and final rel err 9.1e-3 on the fixed-seed
    reference data. The hi part runs 6 full fp16 k-tiles + a 16-row tail;
    the lo part runs 3 fp8e4 DoubleRow pair-matmuls (256 K-rows each at
    ~1.13x a bf16 matmul) + a 16-row fp8 tail. K-tails are replicated at
    partition strips 0/32/64 host-side so the three m-tiles' tail matmuls
    run concurrently in distinct PE row-groups.
  - M=400 tiles as 128+128+128+16; the 16-row remainder is col-strip packed
    4 chunks to a PSUM bank (tile_position strips 0/32/64/96 run
    concurrently). DoubleRow cannot combine with col tiling, so the packed
    m4 block uses plain fp8 (bf16-rate) for its lo part.
  - Layer 2 in fp8e4 DoubleRow: sign() outputs are exactly +-1/0 in e4m3, so
    the matmul is exact and contracts 256 features per instruction. The
    sign() activations write directly into [128,2,512] pair tiles: pair A =
    features (p, 128+p), pair B = (256+p, 384+p) where the second plane is a
    copy of the group's packed m4-sign tile and the unused strips are nulled
    by zero weights (4 per-chunk B-weight variants with rows 384:400 placed
    at the chunk's partition strip).
  - Layer 3 stays bf16, col-strip packed per group of 4 chunks; the final
    group is computed unpacked per-chunk so the last chunk's tail is short.
  - The group loop is software-pipelined: each group's last two layer-2
    calls and its layer-3 pack execute interleaved with the NEXT group's
    layer-1 blocks, so matmuls that consume a Sign activation are issued
    ~8us after the producing PSUM stops -- no activation-handoff stalls.
  - Startup: chunk 0's hi DMA is split so the first k-loop starts ~7us in;
    a short dummy-matmul warm-up keeps the PE busy from ~5.5us so the HAM
    clock gate reaches 2.4 GHz early.
"""

import contextlib
import ctypes
import os
import sys
import types

import numpy as np
import ml_dtypes

import concourse.bacc as bacc
import concourse.mybir as mybir
import concourse.tile as tile
from concourse.bass_utils import run_bass_kernel_spmd


def _ensure_axon_hooks():
    """concourse's trace path imports antenv.axon_hooks, which this image
    lacks; register a ctypes-backed stand-in so trace=True (or a stray
    BASS_TRACE=1 in the environment) cannot crash the run."""
    try:
        import antenv.axon_hooks  # noqa: F401
        return
    except ImportError:
        pass

    so_path = "/opt/axon/libaxon_pjrt.so"
    hook = None
    if os.path.exists(so_path):
        try:
            lib = ctypes.CDLL(so_path)
            if hasattr(lib, "axon_start_nrt_profile"):
                lib.axon_start_nrt_profile.argtypes = [
                    ctypes.POINTER(ctypes.c_int64),
                    ctypes.c_size_t,
                ]
                lib.axon_start_nrt_profile.restype = ctypes.c_int64
                lib.axon_stop_nrt_profile.argtypes = [ctypes.c_char_p]
                lib.axon_stop_nrt_profile.restype = ctypes.c_int64

                @contextlib.contextmanager
                def _hook(output_dir, device_ids):
                    import jax

                    jax.devices()
                    if device_ids:
                        ids = (ctypes.c_int64 * len(device_ids))(*device_ids)
                        rc = lib.axon_start_nrt_profile(ids, len(device_ids))
                    else:
                        rc = lib.axon_start_nrt_profile(None, 0)
                    if rc != 0:
                        raise RuntimeError(f"axon_start_nrt_profile rc={rc}")
                    try:
                        yield
                    finally:
                        lib.axon_stop_nrt_profile(str(output_dir).encode())

                hook = _hook
        except OSError:
            pass

    mod = types.ModuleType("antenv.axon_hooks")
    mod.get_axon_ntff_profile_hook = lambda: hook
    mod.set_axon_ntff_profile_hook = lambda h: None
    sys.modules["antenv.axon_hooks"] = mod

    import concourse.bass_utils as _bu

    _bu.upload_artifacts = lambda tmpdir: tmpdir

BF16 = np.dtype(ml_dtypes.bfloat16)
FP8 = np.dtype(ml_dtypes.float8_e4m3)

NCORES = 8
B = 65536
BL = B // NCORES          # 8192 rows per core
D0, H1, H2, DO = 784, 400, 200, 10
CH = 512                  # batch columns per chunk (PSUM bank = 512 fp32)
NCH = BL // CH            # 16 chunks per core
GRP = 4                   # chunks per packing group
NG = NCH // GRP           # 4 groups
KT = 7                    # 6 full 128-row k-tiles + 16-row tail (K=784)
LSC = 64.0                # lo-residual scale 2^6; weights carry 2^-6
W2PM = 208                # padded M stride for fp8 pair weights (16-aligned)

_cache = {}


def _build():
    if "nc" in _cache:
        return _cache["nc"]

    f32 = mybir.dt.float32
    bf16 = mybir.dt.bfloat16
    f16 = mybir.dt.float16
    fp8 = mybir.dt.float8e4
    u32 = mybir.dt.uint32
    DR = mybir.MatmulPerfMode.DoubleRow
    Sign = mybir.ActivationFunctionType.Sign

    nc = bacc.Bacc("TRN2", debug=False, num_devices=NCORES)

    d_xh = nc.dram_tensor("xh", [NCH, 128, KT, CH], f16, kind="ExternalInput").ap()
    d_xl = nc.dram_tensor("xl", [NCH, 128, KT, CH], fp8, kind="ExternalInput").ap()
    d_w1a = nc.dram_tensor("w1a", [128, KT, 128], f16, kind="ExternalInput").ap()
    d_w1b = nc.dram_tensor("w1b", [128, KT, H1 - 128], f16, kind="ExternalInput").ap()
    d_q1a = nc.dram_tensor("q1a", [128, KT, 128], fp8, kind="ExternalInput").ap()
    d_q1b = nc.dram_tensor("q1b", [128, KT, H1 - 128], fp8, kind="ExternalInput").ap()
    # fp8 DoubleRow pair weights: [:,0] = pair A (rows 0:128 | 128:256);
    # [:,1+jj] = pair B variant for chunk jj (rows 256:384 | rows 384:400 at
    # partition strip 32jj, zero elsewhere).
    d_w2 = nc.dram_tensor("w2p", [128, 5, 2, W2PM], fp8, kind="ExternalInput").ap()
    d_w3 = nc.dram_tensor("w3", [128, 2, DO], bf16, kind="ExternalInput").ap()
    d_out = nc.dram_tensor("out", [NCH, DO, CH], f32, kind="ExternalOutput").ap()

    m2sz = [128, 72]
    k3sz = [128, 72]

    with tile.TileContext(nc) as tc:
        with (
            tc.tile_pool(name="wp", bufs=1) as wp,
            tc.tile_pool(name="xhp", bufs=6) as xhp,
            tc.tile_pool(name="xlp", bufs=6) as xlp,
            tc.tile_pool(name="bap", bufs=4) as bap,
            tc.tile_pool(name="bbp", bufs=4) as bbp,
            tc.tile_pool(name="aqp", bufs=2) as aqp,
            tc.tile_pool(name="a2p", bufs=4) as a2pool,
            tc.tile_pool(name="op", bufs=2) as op,
            tc.tile_pool(name="oup", bufs=4) as oup,
            tc.tile_pool(name="ps1p", bufs=1, space="PSUM") as ps1p,
            tc.tile_pool(name="ps2p", bufs=1, space="PSUM") as ps2p,
            tc.tile_pool(name="pspk", bufs=2, space="PSUM") as pspk,
        ):
            w1a = wp.tile([128, KT, 128], f16, name="w1a")
            w1b = wp.tile([128, KT, H1 - 128], f16, name="w1b")
            q1a = wp.tile([128, KT, 128], fp8, name="q1a")
            q1b = wp.tile([128, KT, H1 - 128], fp8, name="q1b")
            w2sb = wp.tile([128, 5, 2, W2PM], fp8, name="w2sb")
            w3sb = wp.tile([128, 2, DO], bf16, name="w3sb")

            # -- initial DMA issue order: the first layer-1 k-loop needs w1a
            # and the head of chunk 0's hi component, so those go first.
            xh0, xl0 = [], []
            for jj in range(GRP):
                xh0.append(xhp.tile([128, KT, CH], f16, name="xh"))
                xl0.append(xlp.tile([128, KT, CH], fp8, name="xl"))
            nc.sync.dma_start(out=w1a[:], in_=d_w1a)
            nc.sync.dma_start(out=xh0[0][:, 0:2, :], in_=d_xh[0][:, 0:2, :])
            nc.sync.dma_start(out=w1b[:], in_=d_w1b)
            nc.sync.dma_start(out=xh0[0][:, 2:5, :], in_=d_xh[0][:, 2:5, :])
            nc.sync.dma_start(out=xh0[0][:, 5:KT, :], in_=d_xh[0][:, 5:KT, :])
            nc.sync.dma_start(out=q1a[:], in_=d_q1a)
            nc.sync.dma_start(out=q1b[:], in_=d_q1b)
            nc.sync.dma_start(out=xl0[0][:], in_=d_xl[0])
            nc.sync.dma_start(out=xh0[1][:], in_=d_xh[1])
            nc.sync.dma_start(out=xl0[1][:], in_=d_xl[1])
            nc.sync.dma_start(out=w2sb[:], in_=d_w2)
            nc.sync.dma_start(out=w3sb[:], in_=d_w3)
            for jj in (2, 3):
                nc.sync.dma_start(out=xh0[jj][:], in_=d_xh[jj])
                nc.sync.dma_start(out=xl0[jj][:], in_=d_xl[jj])

            def w1_slice(k, m_off, m_sz):
                if m_off == 0:
                    return w1a[:, k, 0:m_sz]
                return w1b[:, k, m_off - 128 : m_off - 128 + m_sz]

            def q1_slice(m_off, m_sz):
                # fp8 lo weights as [128, KT, m] for DoubleRow pair slicing
                if m_off == 0:
                    return q1a[:, :, 0:m_sz]
                return q1b[:, :, m_off - 128 : m_off - 128 + m_sz]

            # HAM/P-state pre-warm: dummy matmuls keep the PE busy during the
            # initial weight/x DMA wait so the clock gate opens early (the
            # activity window is ~3.4us).
            warm = wp.tile([128, 64], f16, name="warm")
            nc.vector.memset(warm[:], 1.0)
            wpss = [
                pspk.tile([64, 64], f32, name=f"wps{i}", tag="pack") for i in range(2)
            ]
            _warm_ctr = [0]

            def warm_fill(n):
                # dummy fp16 matmuls on alternating PSUM banks: keeps the PE
                # busy (HAM activity window) while DMAs land
                for _ in range(n):
                    w = wpss[_warm_ctr[0] & 1]
                    _warm_ctr[0] += 1
                    nc.tensor.matmul(
                        w[:], warm[:, 0:64], warm[:], start=True, stop=True
                    )

            warm_fill(16)

            def layer1_m123(xh, xl, bA, bB, wf=None):
                """Full-width layer-1 m-tiles for one chunk: 6 fp16 hi
                k-tiles + 3 fp8 DoubleRow lo pairs + 16-row hi/lo tails per
                m-tile; Sign() results land in the fp8 pair tiles (bA planes
                0/1, bB plane 0). All fp16 work is issued before all fp8
                work to minimize PE dtype-mode switches.

                The 16-row K-tails are replicated host-side at partition
                strips 0/32/64 so the three m-tiles' tail matmuls run
                concurrently in distinct PE row-groups. wf gives optional
                per-k warm-fill counts for the DMA-limited first chunk."""
                pss = []
                for m in range(3):
                    ps = ps1p.tile(
                        [128, CH], f32, name=f"ps1_{m}", bufs=(2 if m == 0 else 1)
                    )
                    for k in range(KT - 1):
                        if wf is not None and m == 0 and wf[k]:
                            warm_fill(wf[k])
                        nc.tensor.matmul(
                            ps[:],
                            w1_slice(k, m * 128, 128),
                            xh[:, k, :],
                            start=(k == 0),
                            stop=False,
                        )
                    pss.append(ps)
                kl = KT - 1
                for m in range(3):
                    s = 32 * m
                    lhsT = (
                        w1a[s : s + 16, kl, 0:128]
                        if m == 0
                        else w1b[s : s + 16, kl, (m - 1) * 128 : m * 128]
                    )
                    nc.tensor.matmul(
                        pss[m][:],
                        lhsT,
                        xh[s : s + 16, kl, :],
                        start=False,
                        stop=False,
                        tile_position=(s, 0),
                    )
                for m in range(3):
                    q = q1_slice(m * 128, 128)
                    for p in range(3):
                        nc.tensor.matmul(
                            pss[m][:],
                            q[:, 2 * p : 2 * p + 2, :],
                            xl[:, 2 * p : 2 * p + 2, :],
                            start=False,
                            stop=False,
                            perf_mode=DR,
                        )
                for m in range(3):
                    s = 32 * m
                    lhsT = (
                        q1a[s : s + 16, kl, 0:128]
                        if m == 0
                        else q1b[s : s + 16, kl, (m - 1) * 128 : m * 128]
                    )
                    nc.tensor.matmul(
                        pss[m][:],
                        lhsT,
                        xl[s : s + 16, kl, :],
                        start=False,
                        stop=True,
                        tile_position=(s, 0),
                    )
                nc.scalar.activation(bA[:, 0, :], pss[0][:], Sign)
                nc.scalar.activation(bA[:, 1, :], pss[1][:], Sign)
                nc.scalar.activation(bB[:, 0, :], pss[2][:], Sign)

            def m4pack(xhs, xls, bBs):
                """Packed m4 (features 384:400) for a group of 4 chunks:
                col-strip tiled into one PSUM bank (strips run concurrently).
                DoubleRow can't combine with col tiling, so the lo part uses
                plain fp8 matmuls (bf16 rate). Sign -> shared fp8 tile ->
                copied into each chunk's pair-B plane 1 (per-chunk zero
                weights null the other strips)."""
                ps4 = pspk.tile([128, CH], f32, name="ps4", tag="pack")
                nc.vector.memset(ps4[:], 0.0)
                kl = KT - 1
                for k in range(KT - 1):
                    for jj in range(GRP):
                        s = 32 * jj
                        nc.tensor.matmul(
                            ps4[s : s + 16, :],
                            w1b[:, k, 256:272],
                            xhs[jj][:, k, :],
                            start=False,
                            stop=False,
                            tile_position=(0, s),
                        )
                for jj in range(GRP):
                    s = 32 * jj
                    nc.tensor.matmul(
                        ps4[s : s + 16, :],
                        w1b[0:16, kl, 256:272],
                        xhs[jj][0:16, kl, :],
                        start=False,
                        stop=False,
                        tile_position=(0, s),
                    )
                for k in range(KT - 1):
                    for jj in range(GRP):
                        s = 32 * jj
                        nc.tensor.matmul(
                            ps4[s : s + 16, :],
                            q1b[:, k, 256:272],
                            xls[jj][:, k, :],
                            start=False,
                            stop=False,
                            tile_position=(0, s),
                        )
                for jj in range(GRP):
                    s = 32 * jj
                    nc.tensor.matmul(
                        ps4[s : s + 16, :],
                        q1b[0:16, kl, 256:272],
                        xls[jj][0:16, kl, :],
                        start=False,
                        stop=True,
                        tile_position=(0, s),
                    )
                a13q = aqp.tile([128, CH], fp8, name="a13q")
                nc.scalar.activation(a13q[:], ps4[:], Sign)
                for jj in range(GRP):
                    nc.vector.tensor_copy(
                        bBs[jj][:, 1, :].bitcast(u32), a13q[:].bitcast(u32)
                    )

            def layer2(jj, bA, bB):
                """Layer 2 for chunk jj of its group: two fp8e4 DoubleRow
                matmuls per m-tile (pair A then per-chunk pair-B variant),
                exact because all operands are +-1/0."""
                a2 = [None, None]
                for m in range(2):
                    sz = m2sz[m]
                    mo = m * 128
                    ps = ps2p.tile([sz, CH], f32, name=f"ps2_{m}")
                    nc.tensor.matmul(
                        ps[:],
                        w2sb[:, 0, :, mo : mo + sz],
                        bA[:, :, :],
                        start=True,
                        stop=False,
                        perf_mode=DR,
                    )
                    nc.tensor.matmul(
                        ps[:],
                        w2sb[:, 1 + jj, :, mo : mo + sz],
                        bB[:, :, :],
                        start=False,
                        stop=True,
                        perf_mode=DR,
                    )
                    at = a2pool.tile([sz, CH], bf16, name=f"a2_{m}")
                    nc.scalar.activation(at[:], ps[:], Sign)
                    a2[m] = at
                return a2

            def layer3_pack(g, a2s):
                """Layer 3 for a full group, col-strip packed into one PSUM
                bank at strips [32jj : 32jj+10], then one copy + 4 DMAs."""
                ps3 = pspk.tile([128, CH], f32, name="ps3", tag="pack")
                nc.vector.memset(ps3[:], 0.0)
                for k in range(2):
                    ks = k3sz[k]
                    for jj in range(GRP):
                        s = 32 * jj
                        nc.tensor.matmul(
                            ps3[s : s + DO, :],
                            w3sb[0:ks, k, :],
                            a2s[jj][k][0:ks, :],
                            start=False,
                            stop=(k == 1),
                            tile_position=(0, s),
                        )
                osb = op.tile([128, CH], f32, name="osb")
                nc.vector.tensor_copy(osb[:], ps3[:])
                for jj in range(GRP):
                    s = 32 * jj
                    nc.sync.dma_start(
                        out=d_out[g * GRP + jj], in_=osb[s : s + DO, :]
                    )

            def layer3_u(ci, a2):
                """Unpacked per-chunk layer 3 (final group): short
                PSUM -> copy -> DMA tail."""
                ps3u = pspk.tile([16, CH], f32, name="ps3u", tag="pack")
                for k in range(2):
                    ks = k3sz[k]
                    nc.tensor.matmul(
                        ps3u[0:DO, :],
                        w3sb[0:ks, k, 0:DO],
                        a2[k][0:ks, :],
                        start=(k == 0),
                        stop=(k == 1),
                    )
                osbu = oup.tile([16, CH], f32, name="osbu")
                nc.vector.tensor_copy(osbu[0:DO, :], ps3u[0:DO, :])
                nc.sync.dma_start(out=d_out[ci], in_=osbu[0:DO, :])

            # ---- software-pipelined group loop --------------------------
            # steady body(g):  L1(g.0) L2(p.2) L1(g.1) L2(p.3) m4(g)
            #                  L3pack(p) [dma g+1] L1(g.2) L2(g.0)
            #                  L1(g.3) L2(g.1)          (p = g-1)
            xhg, xlg = xh0, xl0
            carry = None  # (g_prev, bAs, bBs, a2s) with L2 of .2/.3 pending
            for g in range(NG):
                bAs = [bap.tile([128, 2, CH], fp8, name="bA") for _ in range(GRP)]
                bBs = [bbp.tile([128, 2, CH], fp8, name="bB") for _ in range(GRP)]
                a2s = [None] * GRP

                # chunk (0,0) is DMA-limited: pad early k-tiles with warm
                # matmuls so the PE never idles long enough to re-throttle
                wf0 = [0, 0, 5, 0, 0, 4] if g == 0 else None
                layer1_m123(xhg[0], xlg[0], bAs[0], bBs[0], wf=wf0)
                if carry is not None:
                    pg, pbA, pbB, pa2 = carry
                    pa2[2] = layer2(2, pbA[2], pbB[2])
                layer1_m123(xhg[1], xlg[1], bAs[1], bBs[1])
                if carry is not None:
                    pa2[3] = layer2(3, pbA[3], pbB[3])
                if g == 0:
                    # group 0: delay m4 until chunk 3's x has landed
                    layer1_m123(xhg[2], xlg[2], bAs[2], bBs[2])
                    m4pack(xhg, xlg, bBs)
                    a2s[0] = layer2(0, bAs[0], bBs[0])
                else:
                    m4pack(xhg, xlg, bBs)
                    layer3_pack(pg, pa2)
                # issue next group's x DMAs (buffers free up as m4 finishes)
                if g + 1 < NG:
                    xhn, xln = [], []
                    for jj in range(GRP):
                        xht = xhp.tile([128, KT, CH], f16, name="xh")
                        xlt = xlp.tile([128, KT, CH], fp8, name="xl")
                        nc.sync.dma_start(out=xht[:], in_=d_xh[(g + 1) * GRP + jj])
                        nc.sync.dma_start(out=xlt[:], in_=d_xl[(g + 1) * GRP + jj])
                        xhn.append(xht)
                        xln.append(xlt)
                if g == 0:
                    layer1_m123(xhg[3], xlg[3], bAs[3], bBs[3])
                    a2s[1] = layer2(1, bAs[1], bBs[1])
                else:
                    layer1_m123(xhg[2], xlg[2], bAs[2], bBs[2])
                    a2s[0] = layer2(0, bAs[0], bBs[0])
                    layer1_m123(xhg[3], xlg[3], bAs[3], bBs[3])
                    a2s[1] = layer2(1, bAs[1], bBs[1])
                carry = (g, bAs, bBs, a2s)
                if g + 1 < NG:
                    xhg, xlg = xhn, xln

            # ---- drain the last group with a short unpacked tail --------
            pg, pbA, pbB, pa2 = carry
            layer3_u(pg * GRP + 0, pa2[0])
            pa2[2] = layer2(2, pbA[2], pbB[2])
            layer3_u(pg * GRP + 1, pa2[1])
            pa2[3] = layer2(3, pbA[3], pbB[3])
            layer3_u(pg * GRP + 2, pa2[2])
            layer3_u(pg * GRP + 3, pa2[3])

    nc.compile()
    _cache["nc"] = nc
    return nc


def _tile7(mat, dtype):
    """[784, N] -> [128, 7, N]: 6 full 128-row k-tiles + 16-row tail
    replicated at partition strips 0/32/64."""
    n = mat.shape[1]
    out = np.zeros((128, KT, n), np.float32)
    for k in range(KT - 1):
        out[:, k, :] = mat[k * 128 : (k + 1) * 128]
    for s in (0, 32, 64):
        out[s : s + 16, KT - 1, :] = mat[768:784]
    return np.ascontiguousarray(out).astype(dtype)


def _prep_weights(W1, W2, W3):
    w1T = np.sign(W1).T.astype(np.float32)  # [784, 400]
    w1h = _tile7(w1T, np.float16)           # [128, 7, 400]
    w1ha = np.ascontiguousarray(w1h[:, :, 0:128])
    w1hb = np.ascontiguousarray(w1h[:, :, 128:H1])
    q1 = _tile7(w1T / LSC, FP8)             # fp8 lo weights (+-2^-6 exact)
    q1a = np.ascontiguousarray(q1[:, :, 0:128])
    q1b = np.ascontiguousarray(q1[:, :, 128:H1])

    # fp8 DoubleRow pair weights for layer 2
    w2T = np.sign(W2).T.astype(np.float32)  # [400, 200]
    w2h = np.zeros((128, 5, 2, W2PM), np.float32)
    w2h[:, 0, 0, 0:H2] = w2T[0:128]
    w2h[:, 0, 1, 0:H2] = w2T[128:256]
    for jj in range(GRP):
        w2h[:, 1 + jj, 0, 0:H2] = w2T[256:384]
        w2h[32 * jj : 32 * jj + 16, 1 + jj, 1, 0:H2] = w2T[384:400]
    w2h = w2h.astype(FP8)

    w3T = np.sign(W3).T.astype(np.float32)  # [200, 10]
    w3h = np.zeros((128, 2, DO), np.float32)
    w3h[:, 0, :] = w3T[0:128]
    w3h[0:72, 1, :] = w3T[128:200]
    w3h = w3h.astype(BF16)
    return w1ha, w1hb, q1a, q1b, w2h, w3h


def _prep_x_core(xc):
    # xc: [8192, 784] fp32 -> hi [16, 128, 7, 512] fp16,
    #                         lo [16, 128, 7, 512] fp8 = e4m3((x - hi) * 2^6)
    xt = np.ascontiguousarray(xc.T.astype(np.float32))  # [784, 8192]
    hi = xt.astype(np.float16)
    lo = (xt - hi.astype(np.float32)) * LSC
    hi7 = _tile7(hi.astype(np.float32), np.float16)     # [128, 7, 8192]
    lo7 = _tile7(lo, FP8)
    xhi = np.ascontiguousarray(
        hi7.reshape(128, KT, NCH, CH).transpose(2, 0, 1, 3)
    )  # [16, 128, 7, 512]
    xlo = np.ascontiguousarray(
        lo7.reshape(128, KT, NCH, CH).transpose(2, 0, 1, 3)
    )
    return xhi, xlo


def kernel(x, W1, W2, W3, _trace=False, **_kw):
    nc = _build()
    w1ha, w1hb, q1a, q1b, w2h, w3h = _prep_weights(
        np.asarray(W1, np.float32), np.asarray(W2, np.float32), np.asarray(W3, np.float32)
    )
    x = np.asarray(x, np.float32).reshape(B, D0)

    in_maps = []
    for c in range(NCORES):
        xhi, xlo = _prep_x_core(x[c * BL : (c + 1) * BL])
        in_maps.append(
            {
                "xh": xhi,
                "xl": xlo,
                "w1a": w1ha,
                "w1b": w1hb,
                "q1a": q1a,
                "q1b": q1b,
                "w2p": w2h,
                "w3": w3h,
            }
        )

    _ensure_axon_hooks()
    res = run_bass_kernel_spmd(nc, in_maps, core_ids=list(range(NCORES)), trace=_trace)

    out = np.empty((B, DO), np.float32)
    for c in range(NCORES):
        oc = res.results[c]["out"]  # [16, 10, 512]
        out[c * BL : (c + 1) * BL] = oc.transpose(0, 2, 1).reshape(BL, DO)
    if _trace:
        _cache["last_results"] = res
    return out


# revision 18
# speedup vs baseline: 1.0199x; 1.0199x over previous
"""Binarized 3-layer MLP on 8 TRN2 NeuronCores (data-parallel over batch).

Computation (matching the reference):
    h1  = x @ sign(W1).T          x: [65536, 784] fp32, W1: [400, 784]
    h2  = sign(h1) @ sign(W2).T   W2: [200, 400]
    out = sign(h2) @ sign(W3).T   W3: [10, 200]

Strategy (v3):
  - Batch sharded 8192 rows/core; weights replicated. Activations kept
    feature-major (features on SBUF partitions) so every layer's contraction
    dim is already on partitions: no transposes.
  - Layer 1 splits x into hi = fp16(x) plus an fp8 residual:
    lo8 = e4m3((x - hi) * 2^6), matched with fp8 weights sign(W1)*2^-6
    (the min-normal e4m3 value, exact). Products are exact power-of-two
    rescales of the residual, accumulated in the same fp32 PSUM group as the
    fp16 hi matmuls, so h1 carries ~2^-15 relative error -- measured 62 h1
    sign flips over 26M entries and final rel err 9.1e-3 on the fixed-seed
    reference data. The hi part runs 6 full fp16 k-tiles + a 16-row tail;
    the lo part runs 3 fp8e4 DoubleRow pair-matmuls (256 K-rows each at
    ~1.13x a bf16 matmul) + a 16-row fp8 tail. K-tails are replicated at
    partition strips 0/32/64 host-side so the three m-tiles' tail matmuls
    run concurrently in distinct PE row-groups.
  - M=400 tiles as 128+128+128+16; the 16-row remainder is col-strip packed
    4 chunks to a PSUM bank (tile_position strips 0/32/64/96 run
    concurrently). DoubleRow cannot combine with col tiling, so the packed
    m4 block uses plain fp8 (bf16-rate) for its lo part.
  - Layer 2 in fp8e4 DoubleRow: sign() outputs are exactly +-1/0 in e4m3, so
    the matmul is exact and contracts 256 features per instruction. The
    sign() activations write directly into [128,2,512] pair tiles: pair A =
    features (p, 128+p), pair B = (256+p, 384+p) where the second plane is a
    copy of the group's packed m4-sign tile and the unused strips are nulled
    by zero weights (4 per-chunk B-weight variants with rows 384:400 placed
    at the chunk's partition strip).
  - Layer 3 stays bf16, col-strip packed per group of 4 chunks; the final
    group is computed unpacked per-chunk so the last chunk's tail is short.
  - The group loop is software-pipelined: each group's last two layer-2
    calls and its layer-3 pack execute interleaved with the NEXT group's
    layer-1 blocks, so matmuls that consume a Sign activation are issued
    ~8us after the producing PSUM stops -- no activation-handoff stalls.
  - Startup: chunk 0's hi DMA is split so the first k-loop starts ~7us in;
    a short dummy-matmul warm-up keeps the PE busy from ~5.5us so the HAM
    clock gate reaches 2.4 GHz early.
"""

import contextlib
import ctypes
import os
import sys
import types

import numpy as np
import ml_dtypes

import concourse.bacc as bacc
import concourse.mybir as mybir
import concourse.tile as tile
from concourse.bass_utils import run_bass_kernel_spmd


def _ensure_axon_hooks():
    """concourse's trace path imports antenv.axon_hooks, which this image
    lacks; register a ctypes-backed stand-in so trace=True (or a stray
    BASS_TRACE=1 in the environment) cannot crash the run."""
    try:
        import antenv.axon_hooks  # noqa: F401
        return
    except ImportError:
        pass

    so_path = "/opt/axon/libaxon_pjrt.so"
    hook = None
    if os.path.exists(so_path):
        try:
            lib = ctypes.CDLL(so_path)
            if hasattr(lib, "axon_start_nrt_profile"):
                lib.axon_start_nrt_profile.argtypes = [
                    ctypes.POINTER(ctypes.c_int64),
                    ctypes.c_size_t,
                ]
                lib.axon_start_nrt_profile.restype = ctypes.c_int64
                lib.axon_stop_nrt_profile.argtypes = [ctypes.c_char_p]
                lib.axon_stop_nrt_profile.restype = ctypes.c_int64

                @contextlib.contextmanager
                def _hook(output_dir, device_ids):
                    import jax

                    jax.devices()
                    if device_ids:
                        ids = (ctypes.c_int64 * len(device_ids))(*device_ids)
                        rc = lib.axon_start_nrt_profile(ids, len(device_ids))
                    else:
                        rc = lib.axon_start_nrt_profile(None, 0)
                    if rc != 0:
                        raise RuntimeError(f"axon_start_nrt_profile rc={rc}")
                    try:
                        yield
                    finally:
                        lib.axon_stop_nrt_profile(str(output_dir).encode())

                hook = _hook
        except OSError:
            pass

    mod = types.ModuleType("antenv.axon_hooks")
    mod.get_axon_ntff_profile_hook = lambda: hook
    mod.set_axon_ntff_profile_hook = lambda h: None
    sys.modules["antenv.axon_hooks"] = mod

    import concourse.bass_utils as _bu

    _bu.upload_artifacts = lambda tmpdir: tmpdir

BF16 = np.dtype(ml_dtypes.bfloat16)
FP8 = np.dtype(ml_dtypes.float8_e4m3)

NCORES = 8
B = 65536
BL = B // NCORES          # 8192 rows per core
D0, H1, H2, DO = 784, 400, 200, 10
CH = 512                  # batch columns per chunk (PSUM bank = 512 fp32)
NCH = BL // CH            # 16 chunks per core
GRP = 4                   # chunks per packing group
NG = NCH // GRP           # 4 groups
KT = 7                    # 6 full 128-row k-tiles + 16-row tail (K=784)
LSC = 64.0                # lo-residual scale 2^6; weights carry 2^-6
W2PM = 208                # padded M stride for fp8 pair weights (16-aligned)

_cache = {}


def _build():
    if "nc" in _cache:
        return _cache["nc"]

    f32 = mybir.dt.float32
    bf16 = mybir.dt.bfloat16
    f16 = mybir.dt.float16
    fp8 = mybir.dt.float8e4
    u32 = mybir.dt.uint32
    DR = mybir.MatmulPerfMode.DoubleRow
    Sign = mybir.ActivationFunctionType.Sign

    nc = bacc.Bacc("TRN2", debug=False, num_devices=NCORES)

    d_xh = nc.dram_tensor("xh", [NCH, 128, KT, CH], f16, kind="ExternalInput").ap()
    d_xl = nc.dram_tensor("xl", [NCH, 128, KT, CH], fp8, kind="ExternalInput").ap()
    d_w1a = nc.dram_tensor("w1a", [128, KT, 128], f16, kind="ExternalInput").ap()
    d_w1b = nc.dram_tensor("w1b", [128, KT, H1 - 128], f16, kind="ExternalInput").ap()
    d_q1a = nc.dram_tensor("q1a", [128, KT, 128], fp8, kind="ExternalInput").ap()
    d_q1b = nc.dram_tensor("q1b", [128, KT, H1 - 128], fp8, kind="ExternalInput").ap()
    # fp8 DoubleRow pair weights: [:,0] = pair A (rows 0:128 | 128:256);
    # [:,1+jj] = pair B variant for chunk jj (rows 256:384 | rows 384:400 at
    # partition strip 32jj, zero elsewhere).
    d_w2 = nc.dram_tensor("w2p", [128, 5, 2, W2PM], fp8, kind="ExternalInput").ap()
    d_w3 = nc.dram_tensor("w3", [128, 2, DO], bf16, kind="ExternalInput").ap()
    d_out = nc.dram_tensor("out", [NCH, DO, CH], f32, kind="ExternalOutput").ap()

    m2sz = [128, 72]
    k3sz = [128, 72]

    with tile.TileContext(nc) as tc:
        with (
            tc.tile_pool(name="wp", bufs=1) as wp,
            tc.tile_pool(name="xhp", bufs=6) as xhp,
            tc.tile_pool(name="xlp", bufs=6) as xlp,
            tc.tile_pool(name="bap", bufs=4) as bap,
            tc.tile_pool(name="bbp", bufs=4) as bbp,
            tc.tile_pool(name="aqp", bufs=2) as aqp,
            tc.tile_pool(name="a2p", bufs=4) as a2pool,
            tc.tile_pool(name="op", bufs=2) as op,
            tc.tile_pool(name="oup", bufs=4) as oup,
            tc.tile_pool(name="ps1p", bufs=1, space="PSUM") as ps1p,
            tc.tile_pool(name="ps2p", bufs=1, space="PSUM") as ps2p,
            tc.tile_pool(name="pspk", bufs=2, space="PSUM") as pspk,
        ):
            w1a = wp.tile([128, KT, 128], f16, name="w1a")
            w1b = wp.tile([128, KT, H1 - 128], f16, name="w1b")
            q1a = wp.tile([128, KT, 128], fp8, name="q1a")
            q1b = wp.tile([128, KT, H1 - 128], fp8, name="q1b")
            w2sb = wp.tile([128, 5, 2, W2PM], fp8, name="w2sb")
            w3sb = wp.tile([128, 2, DO], bf16, name="w3sb")

            # -- initial DMA issue order: the first layer-1 k-loop needs w1a
            # and the head of chunk 0's hi component, so those go first.
            xh0, xl0 = [], []
            for jj in range(GRP):
                xh0.append(xhp.tile([128, KT, CH], f16, name="xh"))
                xl0.append(xlp.tile([128, KT, CH], fp8, name="xl"))
            nc.sync.dma_start(out=w1a[:], in_=d_w1a)
            nc.sync.dma_start(out=xh0[0][:, 0:3, :], in_=d_xh[0][:, 0:3, :])
            nc.sync.dma_start(out=w1b[:], in_=d_w1b)
            nc.sync.dma_start(out=xh0[0][:, 3:KT, :], in_=d_xh[0][:, 3:KT, :])
            nc.sync.dma_start(out=q1a[:], in_=d_q1a)
            nc.sync.dma_start(out=q1b[:], in_=d_q1b)
            nc.sync.dma_start(out=xl0[0][:], in_=d_xl[0])
            nc.sync.dma_start(out=xh0[1][:], in_=d_xh[1])
            nc.sync.dma_start(out=xl0[1][:], in_=d_xl[1])
            nc.sync.dma_start(out=w2sb[:], in_=d_w2)
            nc.sync.dma_start(out=w3sb[:], in_=d_w3)
            for jj in (2, 3):
                nc.sync.dma_start(out=xh0[jj][:], in_=d_xh[jj])
                nc.sync.dma_start(out=xl0[jj][:], in_=d_xl[jj])

            def w1_slice(k, m_off, m_sz):
                if m_off == 0:
                    return w1a[:, k, 0:m_sz]
                return w1b[:, k, m_off - 128 : m_off - 128 + m_sz]

            def q1_slice(m_off, m_sz):
                # fp8 lo weights as [128, KT, m] for DoubleRow pair slicing
                if m_off == 0:
                    return q1a[:, :, 0:m_sz]
                return q1b[:, :, m_off - 128 : m_off - 128 + m_sz]

            # HAM/P-state pre-warm: dummy matmuls keep the PE busy during the
            # initial weight/x DMA wait so the clock gate opens early (the
            # activity window is ~3.4us).
            # HAM pre-warm: full-width 512-col dummy matmuls give a 100%-duty
            # PE stream from ~5.5us, so the clock gate's fully-busy activity
            # window opens (2.4 GHz) before the first real matmuls run.
            warm = wp.tile([128, CH], f16, name="warm")
            nc.vector.memset(warm[:], 1.0)
            wpss = [
                pspk.tile([64, CH], f32, name=f"wps{i}", tag="pack") for i in range(2)
            ]
            for i in range(10):
                nc.tensor.matmul(
                    wpss[i & 1][:], warm[:, 0:64], warm[:], start=True, stop=True
                )

            def layer1_m123(xh, xl, bA, bB):
                """Full-width layer-1 m-tiles for one chunk: 6 fp16 hi
                k-tiles + 3 fp8 DoubleRow lo pairs + 16-row hi/lo tails per
                m-tile; Sign() results land in the fp8 pair tiles (bA planes
                0/1, bB plane 0). All fp16 work is issued before all fp8
                work to minimize PE dtype-mode switches.

                The 16-row K-tails are replicated host-side at partition
                strips 0/32/64 so the three m-tiles' tail matmuls run
                concurrently in distinct PE row-groups."""
                pss = []
                for m in range(3):
                    ps = ps1p.tile(
                        [128, CH], f32, name=f"ps1_{m}", bufs=(2 if m == 0 else 1)
                    )
                    for k in range(KT - 1):
                        nc.tensor.matmul(
                            ps[:],
                            w1_slice(k, m * 128, 128),
                            xh[:, k, :],
                            start=(k == 0),
                            stop=False,
                        )
                    pss.append(ps)
                kl = KT - 1
                for m in range(3):
                    s = 32 * m
                    lhsT = (
                        w1a[s : s + 16, kl, 0:128]
                        if m == 0
                        else w1b[s : s + 16, kl, (m - 1) * 128 : m * 128]
                    )
                    nc.tensor.matmul(
                        pss[m][:],
                        lhsT,
                        xh[s : s + 16, kl, :],
                        start=False,
                        stop=False,
                        tile_position=(s, 0),
                    )
                for m in range(3):
                    q = q1_slice(m * 128, 128)
                    for p in range(3):
                        nc.tensor.matmul(
                            pss[m][:],
                            q[:, 2 * p : 2 * p + 2, :],
                            xl[:, 2 * p : 2 * p + 2, :],
                            start=False,
                            stop=False,
                            perf_mode=DR,
                        )
                for m in range(3):
                    s = 32 * m
                    lhsT = (
                        q1a[s : s + 16, kl, 0:128]
                        if m == 0
                        else q1b[s : s + 16, kl, (m - 1) * 128 : m * 128]
                    )
                    nc.tensor.matmul(
                        pss[m][:],
                        lhsT,
                        xl[s : s + 16, kl, :],
                        start=False,
                        stop=True,
                        tile_position=(s, 0),
                    )
                nc.scalar.activation(bA[:, 0, :], pss[0][:], Sign)
                nc.scalar.activation(bA[:, 1, :], pss[1][:], Sign)
                nc.scalar.activation(bB[:, 0, :], pss[2][:], Sign)

            def m4pack(xhs, xls, bBs):
                """Packed m4 (features 384:400) for a group of 4 chunks:
                col-strip tiled into one PSUM bank (strips run concurrently).
                DoubleRow can't combine with col tiling, so the lo part uses
                plain fp8 matmuls (bf16 rate). Sign -> shared fp8 tile ->
                copied into each chunk's pair-B plane 1 (per-chunk zero
                weights null the other strips)."""
                ps4 = pspk.tile([128, CH], f32, name="ps4", tag="pack")
                nc.vector.memset(ps4[:], 0.0)
                kl = KT - 1
                for k in range(KT - 1):
                    for jj in range(GRP):
                        s = 32 * jj
                        nc.tensor.matmul(
                            ps4[s : s + 16, :],
                            w1b[:, k, 256:272],
                            xhs[jj][:, k, :],
                            start=False,
                            stop=False,
                            tile_position=(0, s),
                        )
                for jj in range(GRP):
                    s = 32 * jj
                    nc.tensor.matmul(
                        ps4[s : s + 16, :],
                        w1b[0:16, kl, 256:272],
                        xhs[jj][0:16, kl, :],
                        start=False,
                        stop=False,
                        tile_position=(0, s),
                    )
                for k in range(KT - 1):
                    for jj in range(GRP):
                        s = 32 * jj
                        nc.tensor.matmul(
                            ps4[s : s + 16, :],
                            q1b[:, k, 256:272],
                            xls[jj][:, k, :],
                            start=False,
                            stop=False,
                            tile_position=(0, s),
                        )
                for jj in range(GRP):
                    s = 32 * jj
                    nc.tensor.matmul(
                        ps4[s : s + 16, :],
                        q1b[0:16, kl, 256:272],
                        xls[jj][0:16, kl, :],
                        start=False,
                        stop=True,
                        tile_position=(0, s),
                    )
                a13q = aqp.tile([128, CH], fp8, name="a13q")
                nc.scalar.activation(a13q[:], ps4[:], Sign)
                for jj in range(GRP):
                    nc.vector.tensor_copy(
                        bBs[jj][:, 1, :].bitcast(u32), a13q[:].bitcast(u32)
                    )

            def layer2(jj, bA, bB):
                """Layer 2 for chunk jj of its group: two fp8e4 DoubleRow
                matmuls per m-tile (pair A then per-chunk pair-B variant),
                exact because all operands are +-1/0."""
                a2 = [None, None]
                for m in range(2):
                    sz = m2sz[m]
                    mo = m * 128
                    ps = ps2p.tile([sz, CH], f32, name=f"ps2_{m}")
                    nc.tensor.matmul(
                        ps[:],
                        w2sb[:, 0, :, mo : mo + sz],
                        bA[:, :, :],
                        start=True,
                        stop=False,
                        perf_mode=DR,
                    )
                    nc.tensor.matmul(
                        ps[:],
                        w2sb[:, 1 + jj, :, mo : mo + sz],
                        bB[:, :, :],
                        start=False,
                        stop=True,
                        perf_mode=DR,
                    )
                    at = a2pool.tile([sz, CH], bf16, name=f"a2_{m}")
                    nc.scalar.activation(at[:], ps[:], Sign)
                    a2[m] = at
                return a2

            def layer3_pack(g, a2s):
                """Layer 3 for a full group, col-strip packed into one PSUM
                bank at strips [32jj : 32jj+10], then one copy + 4 DMAs."""
                ps3 = pspk.tile([128, CH], f32, name="ps3", tag="pack")
                nc.vector.memset(ps3[:], 0.0)
                for k in range(2):
                    ks = k3sz[k]
                    for jj in range(GRP):
                        s = 32 * jj
                        nc.tensor.matmul(
                            ps3[s : s + DO, :],
                            w3sb[0:ks, k, :],
                            a2s[jj][k][0:ks, :],
                            start=False,
                            stop=(k == 1),
                            tile_position=(0, s),
                        )
                osb = op.tile([128, CH], f32, name="osb")
                nc.vector.tensor_copy(osb[:], ps3[:])
                for jj in range(GRP):
                    s = 32 * jj
                    nc.sync.dma_start(
                        out=d_out[g * GRP + jj], in_=osb[s : s + DO, :]
                    )

            def layer3_u(ci, a2):
                """Unpacked per-chunk layer 3 (final group): short
                PSUM -> copy -> DMA tail."""
                ps3u = pspk.tile([16, CH], f32, name="ps3u", tag="pack")
                for k in range(2):
                    ks = k3sz[k]
                    nc.tensor.matmul(
                        ps3u[0:DO, :],
                        w3sb[0:ks, k, 0:DO],
                        a2[k][0:ks, :],
                        start=(k == 0),
                        stop=(k == 1),
                    )
                osbu = oup.tile([16, CH], f32, name="osbu")
                nc.vector.tensor_copy(osbu[0:DO, :], ps3u[0:DO, :])
                nc.sync.dma_start(out=d_out[ci], in_=osbu[0:DO, :])

            # ---- software-pipelined group loop --------------------------
            # steady body(g):  L1(g.0) L2(p.2) L1(g.1) L2(p.3) m4(g)
            #                  L3pack(p) [dma g+1] L1(g.2) L2(g.0)
            #                  L1(g.3) L2(g.1)          (p = g-1)
            xhg, xlg = xh0, xl0
            carry = None  # (g_prev, bAs, bBs, a2s) with L2 of .2/.3 pending
            for g in range(NG):
                bAs = [bap.tile([128, 2, CH], fp8, name="bA") for _ in range(GRP)]
                bBs = [bbp.tile([128, 2, CH], fp8, name="bB") for _ in range(GRP)]
                a2s = [None] * GRP

                layer1_m123(xhg[0], xlg[0], bAs[0], bBs[0])
                if carry is not None:
                    pg, pbA, pbB, pa2 = carry
                    pa2[2] = layer2(2, pbA[2], pbB[2])
                layer1_m123(xhg[1], xlg[1], bAs[1], bBs[1])
                if carry is not None:
                    pa2[3] = layer2(3, pbA[3], pbB[3])
                if g == 0:
                    # group 0: delay m4 until chunk 3's x has landed
                    layer1_m123(xhg[2], xlg[2], bAs[2], bBs[2])
                    m4pack(xhg, xlg, bBs)
                    a2s[0] = layer2(0, bAs[0], bBs[0])
                else:
                    m4pack(xhg, xlg, bBs)
                    layer3_pack(pg, pa2)
                # issue next group's x DMAs (buffers free up as m4 finishes)
                if g + 1 < NG:
                    xhn, xln = [], []
                    for jj in range(GRP):
                        xht = xhp.tile([128, KT, CH], f16, name="xh")
                        xlt = xlp.tile([128, KT, CH], fp8, name="xl")
                        nc.sync.dma_start(out=xht[:], in_=d_xh[(g + 1) * GRP + jj])
                        nc.sync.dma_start(out=xlt[:], in_=d_xl[(g + 1) * GRP + jj])
                        xhn.append(xht)
                        xln.append(xlt)
                if g == 0:
                    layer1_m123(xhg[3], xlg[3], bAs[3], bBs[3])
                    a2s[1] = layer2(1, bAs[1], bBs[1])
                else:
                    layer1_m123(xhg[2], xlg[2], bAs[2], bBs[2])
                    a2s[0] = layer2(0, bAs[0], bBs[0])
                    layer1_m123(xhg[3], xlg[3], bAs[3], bBs[3])
                    a2s[1] = layer2(1, bAs[1], bBs[1])
                carry = (g, bAs, bBs, a2s)
                if g + 1 < NG:
                    xhg, xlg = xhn, xln

            # ---- drain the last group with a short unpacked tail --------
            pg, pbA, pbB, pa2 = carry
            layer3_u(pg * GRP + 0, pa2[0])
            pa2[2] = layer2(2, pbA[2], pbB[2])
            layer3_u(pg * GRP + 1, pa2[1])
            pa2[3] = layer2(3, pbA[3], pbB[3])
            layer3_u(pg * GRP + 2, pa2[2])
            layer3_u(pg * GRP + 3, pa2[3])

    nc.compile()
    _cache["nc"] = nc
    return nc


def _tile7(mat, dtype):
    """[784, N] -> [128, 7, N]: 6 full 128-row k-tiles + 16-row tail
    replicated at partition strips 0/32/64."""
    n = mat.shape[1]
    out = np.zeros((128, KT, n), np.float32)
    for k in range(KT - 1):
        out[:, k, :] = mat[k * 128 : (k + 1) * 128]
    for s in (0, 32, 64):
        out[s : s + 16, KT - 1, :] = mat[768:784]
    return np.ascontiguousarray(out).astype(dtype)


def _prep_weights(W1, W2, W3):
    w1T = np.sign(W1).T.astype(np.float32)  # [784, 400]
    w1h = _tile7(w1T, np.float16)           # [128, 7, 400]
    w1ha = np.ascontiguousarray(w1h[:, :, 0:128])
    w1hb = np.ascontiguousarray(w1h[:, :, 128:H1])
    q1 = _tile7(w1T / LSC, FP8)             # fp8 lo weights (+-2^-6 exact)
    q1a = np.ascontiguousarray(q1[:, :, 0:128])
    q1b = np.ascontiguousarray(q1[:, :, 128:H1])

    # fp8 DoubleRow pair weights for layer 2
    w2T = np.sign(W2).T.astype(np.float32)  # [400, 200]
    w2h = np.zeros((128, 5, 2, W2PM), np.float32)
    w2h[:, 0, 0, 0:H2] = w2T[0:128]
    w2h[:, 0, 1, 0:H2] = w2T[128:256]
    for jj in range(GRP):
        w2h[:, 1 + jj, 0, 0:H2] = w2T[256:384]
        w2h[32 * jj : 32 * jj + 16, 1 + jj, 1, 0:H2] = w2T[384:400]
    w2h = w2h.astype(FP8)

    w3T = np.sign(W3).T.astype(np.float32)  # [200, 10]
    w3h = np.zeros((128, 2, DO), np.float32)
    w3h[:, 0, :] = w3T[0:128]
    w3h[0:72, 1, :] = w3T[128:200]
    w3h = w3h.astype(BF16)
    return w1ha, w1hb, q1a, q1b, w2h, w3h


def _prep_x_core(xc):
    # xc: [8192, 784] fp32 -> hi [16, 128, 7, 512] fp16,
    #                         lo [16, 128, 7, 512] fp8 = e4m3((x - hi) * 2^6)
    xt = np.ascontiguousarray(xc.T.astype(np.float32))  # [784, 8192]
    hi = xt.astype(np.float16)
    lo = (xt - hi.astype(np.float32)) * LSC
    hi7 = _tile7(hi.astype(np.float32), np.float16)     # [128, 7, 8192]
    lo7 = _tile7(lo, FP8)
    xhi = np.ascontiguousarray(
        hi7.reshape(128, KT, NCH, CH).transpose(2, 0, 1, 3)
    )  # [16, 128, 7, 512]
    xlo = np.ascontiguousarray(
        lo7.reshape(128, KT, NCH, CH).transpose(2, 0, 1, 3)
    )
    return xhi, xlo


def kernel(x, W1, W2, W3, _trace=False, **_kw):
    nc = _build()
    w1ha, w1hb, q1a, q1b, w2h, w3h = _prep_weights(
        np.asarray(W1, np.float32), np.asarray(W2, np.float32), np.asarray(W3, np.float32)
    )
    x = np.asarray(x, np.float32).reshape(B, D0)

    in_maps = []
    for c in range(NCORES):
        xhi, xlo = _prep_x_core(x[c * BL : (c + 1) * BL])
        in_maps.append(
            {
                "xh": xhi,
                "xl": xlo,
                "w1a": w1ha,
                "w1b": w1hb,
                "q1a": q1a,
                "q1b": q1b,
                "w2p": w2h,
                "w3": w3h,
            }
        )

    _ensure_axon_hooks()
    res = run_bass_kernel_spmd(nc, in_maps, core_ids=list(range(NCORES)), trace=_trace)

    out = np.empty((B, DO), np.float32)
    for c in range(NCORES):
        oc = res.results[c]["out"]  # [16, 10, 512]
        out[c * BL : (c + 1) * BL] = oc.transpose(0, 2, 1).reshape(BL, DO)
    if _trace:
        _cache["last_results"] = res
    return out
